# revision 8
# baseline (speedup 1.0000x reference)
"""DispNetC correlation volume on 8 NeuronCores (Trainium2, Bass/Tile).

out[b, d, h, w] = mean_c(L[b,c,h,w] * R[b,c,h,w-d]), d in [0,40), 0 where w<d.

Sharding: data-parallel over batch (B=8 -> 1 sample per core). Per core:

1. Load L, R into SBUF as [c_lo(128 part), (c_hi, h, w)] in NSPLIT chunks.
2. Per h: 2 accumulating fp32 matmuls -> PSUM Gram G[w, w'] = sum_c L[c,w]R[c,w'].
   The needed band is out[d, w] = G[w, w-d]/256 - 40 diagonals, which are
   partition-coupled in [w, w'] layout (no legal AP reads a diagonal).
3. Re-layout to h-on-partition form via one DRAM round trip:
   DVE-copy Grams into G_all[w, h*128 + w'], one DMA G_all -> scratch[w, h, w']
   (+GUARD prefix for w' < 0 reads), two DMAs back as
   X[p = 64q + h, i*103 + j] holding G[w = 64q + i, w' = 64q - 39 + j]
   (clipped to valid w'; X is pre-zeroed so w' < 0 holes = the w<d zeros).
4. In X a diagonal d for ALL h is an uncoupled strided AP: element
   (p, i, j=i+39-d) sits at free offset 104*i + 39 - d ->
   one DVE scalar-mul (x 1/256) per d -> O[p, 64*d + i].
5. Two DMAs (one per q) write O to out[d, h, w] in 512B-contiguous runs.

DMA count is minimized aggressively: this environment shows ~14us fixed cost
per DMA instruction, dominating everything else.
"""

import numpy as np

C, H, W, D = 256, 64, 128, 40
NS = 103                 # per-q window width (39 + 64)
XF = 64 * NS             # X free size
F3 = D * 64              # O free size
GUARD = 64               # scratch guard elems for w' < 0 reads
N_CORES = 8
NSPLIT = 2               # input load chunks per tensor (h-blocks)

_cache = {}


def _build(n_cores=N_CORES, nsplit=NSPLIT):
    import concourse.bass as bass
    import concourse.bacc as bacc
    import concourse.mybir as mybir
    from concourse.tile import TileContext

    f32 = mybir.dt.float32
    nc = bacc.Bacc("TRN2", target_bir_lowering=False, debug=False,
                   num_devices=n_cores)
    l_in = nc.dram_tensor("l", [C, H, W], f32, kind="ExternalInput")
    r_in = nc.dram_tensor("r", [C, H, W], f32, kind="ExternalInput")
    out = nc.dram_tensor("out", [D, H, W], f32, kind="ExternalOutput")

    HBLK = H // nsplit

    with TileContext(nc) as tc:
        with (
            tc.tile_pool(name="inp", bufs=2) as inp,
            tc.tile_pool(name="fix", bufs=1) as fix,
            tc.tile_pool(name="ps", bufs=6, space="PSUM") as psp,
            tc.tile_pool(name="dram", bufs=1, space="DRAM") as dp,
        ):
            g_all = fix.tile([128, H * W], f32, tag="gall")
            ga3 = g_all[:, :].rearrange("w (h x) -> w h x", x=W)
            x_t = fix.tile([128, XF], f32, tag="x")
            o_t = fix.tile([128, F3], f32, tag="o")
            scratch = dp.tile([GUARD + 128 * H * W], f32)
            sflat = scratch[:]

            lv = l_in.ap().rearrange("(ch p) h w -> p ch h w", ch=2)
            rv = r_in.ap().rearrange("(ch p) h w -> p ch h w", ch=2)

            for blk in range(nsplit):
                h0 = blk * HBLK
                lt = inp.tile([128, 2 * HBLK * W], f32, tag="lt")
                rt = inp.tile([128, 2 * HBLK * W], f32, tag="rt")
                lt4 = lt[:, :].rearrange("p (ch h w) -> p ch h w", ch=2, h=HBLK)
                rt4 = rt[:, :].rearrange("p (ch h w) -> p ch h w", ch=2, h=HBLK)
                nc.sync.dma_start(lt4, lv[:, :, h0 : h0 + HBLK, :])
                nc.scalar.dma_start(rt4, rv[:, :, h0 : h0 + HBLK, :])
                for hb in range(HBLK):
                    h = h0 + hb
                    gm = psp.tile([128, W], f32, tag="gram")
                    for ch in range(2):
                        nc.tensor.matmul(
                            gm[:, :], lt4[:, ch, hb, :], rt4[:, ch, hb, :],
                            start=(ch == 0), stop=(ch == 1),
                        )
                    nc.vector.tensor_copy(ga3[:, h, :], gm[:, :])

            # G_all -> DRAM scratch (one DMA): scratch[GUARD + w*H*W + h*W + w']
            sc3 = sflat[GUARD:].rearrange("(w h x) -> w h x", w=128, h=H)
            nc.sync.dma_start(sc3, ga3)

            # baseline-zero X (covers the q=0 j<39 hole = w<d zeros, and
            # keeps CoreSim's interval-based init tracking happy)
            nc.vector.memset(x_t[:, :], 0.0)

            # readback per q: X[64q+h, i*103+j] <- scratch[w=64q+i, h, w']
            sc4 = sflat[GUARD:].rearrange("(i h x) -> i h x", i=128, h=H)
            for q in range(2):
                j0 = 39 if q == 0 else 0
                wlo = 64 * q - 39 + j0
                src_ap = sc4[64 * q : 64 * q + 64, :, wlo : wlo + NS - j0]
                dst = x_t[64 * q : 64 * q + 64, :].rearrange(
                    "h (i j) -> h i j", j=NS)[:, :, j0:]
                eng = nc.sync if q == 0 else nc.scalar
                eng.dma_start(dst, src_ap.transpose([1, 0, 2]))

            # per-diagonal extraction with 1/C scale
            xs = x_t[:, :]
            ovw = o_t[:, :].rearrange("p (d i) -> p d i", d=D)
            for d in range(D):
                lo = 39 - d
                nc.vector.tensor_scalar_mul(
                    ovw[:, d, :],
                    xs[:, lo : lo + 104 * 63 + 1 : 104],
                    1.0 / C,
                )

            # out DMAs: one per q, contiguous partitions [64q, 64q+64)
            dstq = out.ap().rearrange("d h (two w) -> two h d w", two=2)
            for q in range(2):
                srcq = o_t[64 * q : 64 * q + 64, :].rearrange(
                    "h (d w) -> h d w", d=D)
                eng = nc.sync if q == 0 else nc.scalar
                eng.dma_start(dstq[q], srcq)

    nc.compile()
    return nc


def _get_program():
    if "nc" not in _cache:
        _cache["nc"] = _build()
    return _cache["nc"]


def kernel(conv3a_l: np.ndarray, conv3a_r: np.ndarray) -> np.ndarray:
    from concourse import bass_utils

    nc = _get_program()
    conv3a_l = np.ascontiguousarray(conv3a_l, dtype=np.float32)
    conv3a_r = np.ascontiguousarray(conv3a_r, dtype=np.float32)
    in_maps = [
        {"l": conv3a_l[b], "r": conv3a_r[b]} for b in range(N_CORES)
    ]
    res = bass_utils.run_bass_kernel_spmd(nc, in_maps,
                                          core_ids=list(range(N_CORES)))
    return np.stack([res.results[b]["out"] for b in range(N_CORES)], axis=0)
